# revision 3
# baseline (speedup 1.0000x reference)
"""Trainium2 Bass kernel for nn_AttentionHead (B=16, S=2048, D=1024, H=128).

Single attention head with key-mask + deterministic dropout (jax key 42).
Data-parallel over batch: 2 batch elements per core on 8 cores.

Per-core layout strategy (all matmuls contract over the partition dim):
  - x is pre-transposed on host to xT [D, S] so QKV projections run directly.
  - Q^T, K^T produced as [H=128 part, S free]; V produced as [S part, H free].
  - scores^T [k part, q free] = K_blk @ Q^T  (one 128-contraction matmul).
  - attn mask folds into exp's per-partition bias (-1e9 on masked k).
  - E = exp(scores^T + bias); denominator = ones-matmul over E (PSUM f32).
  - dropout = precomputed {0, 1/(1-p)} bf16 mask (elementwise on DVE).
  - out^T [H, q] = sum_k V_blk^T @ (E*mask); transpose + per-q 1/denom scale.
"""

import math

import numpy as np
import ml_dtypes

B, S, D, H = 16, 2048, 1024, 128
N_CORES = 8
BPC = B // N_CORES  # batches per core
QB = 512            # query block (free dim per matmul)
NQB = S // QB
NKB = S // 128      # key blocks of 128
DROPOUT_P = 0.3
QSCALE = 1.0 / math.sqrt(H)

_BF16 = ml_dtypes.bfloat16


def _build_bass():
    import concourse.mybir as mybir
    import concourse.tile as tile
    from concourse import bacc
    from concourse.masks import make_identity

    f32 = mybir.dt.float32
    bf16 = mybir.dt.bfloat16
    AF = mybir.ActivationFunctionType

    nc = bacc.Bacc(None, target_bir_lowering=False, debug=False)

    xT = nc.dram_tensor("xT", [BPC, D, S], bf16, kind="ExternalInput")
    qwT = nc.dram_tensor("qwT", [D, H], bf16, kind="ExternalInput")
    kwT = nc.dram_tensor("kwT", [D, H], bf16, kind="ExternalInput")
    vwT = nc.dram_tensor("vwT", [D, H], bf16, kind="ExternalInput")
    qb2 = nc.dram_tensor("qb2", [H, 1], f32, kind="ExternalInput")  # pre-scaled
    kb2 = nc.dram_tensor("kb2", [H, 1], f32, kind="ExternalInput")
    vb2 = nc.dram_tensor("vb2", [1, H], bf16, kind="ExternalInput")
    biask = nc.dram_tensor("biask", [BPC, 128, NKB], f32, kind="ExternalInput")
    maskT = nc.dram_tensor("maskT", [BPC, S, S], bf16, kind="ExternalInput")
    out = nc.dram_tensor("out", [BPC, S, H], f32, kind="ExternalOutput")

    with tile.TileContext(nc) as tc:
        with (
            tc.tile_pool(name="singles", bufs=1) as singles,
            tc.tile_pool(name="work", bufs=4) as work,
            tc.tile_pool(name="mwork", bufs=8) as mwork,
            tc.tile_pool(name="ep", bufs=2) as ep,
            tc.tile_pool(name="psum_mm", bufs=3, space="PSUM") as psum_mm,
            tc.tile_pool(name="psum_acc", bufs=1, space="PSUM") as psum_acc,
            tc.tile_pool(name="psum_tp", bufs=2, space="PSUM") as psum_tp,
        ):
            # ---- constants / weights ----
            ident = singles.tile([128, 128], f32)
            make_identity(nc, ident)
            ones = singles.tile([128, 128], bf16)
            nc.vector.memset(ones, 1.0)

            qw_sb = singles.tile([128, D // 128, H], bf16)
            kw_sb = singles.tile([128, D // 128, H], bf16)
            vw_sb = singles.tile([128, D // 128, H], bf16)
            nc.sync.dma_start(qw_sb, qwT.rearrange("(c p) h -> p c h", p=128))
            nc.sync.dma_start(kw_sb, kwT.rearrange("(c p) h -> p c h", p=128))
            nc.sync.dma_start(vw_sb, vwT.rearrange("(c p) h -> p c h", p=128))

            qb_sb = singles.tile([128, 1], f32)
            kb_sb = singles.tile([128, 1], f32)
            nc.sync.dma_start(qb_sb, qb2[:, :])
            nc.sync.dma_start(kb_sb, kb2[:, :])
            vb_sb = singles.tile([128, H], bf16)
            nc.sync.dma_start(vb_sb, vb2[:, :].to_broadcast((128, H)))

            bias_sb = singles.tile([128, BPC, NKB], f32)
            for b in range(BPC):
                nc.sync.dma_start(bias_sb[:, b, :], biask[b])

            # ---- x^T resident in SBUF: [128, BPC, D/128, S] bf16 ----
            xts = singles.tile([128, BPC, D // 128, S], bf16)
            for b in range(BPC):
                for c in range(D // 128):
                    nc.sync.dma_start(
                        xts[:, b, c, :], xT[b, c * 128 : (c + 1) * 128, :]
                    )

            QT = singles.tile([128, BPC, S], bf16)  # [H, q]
            KT = singles.tile([128, BPC, S], bf16)  # [H, k]
            Vs = singles.tile([128, BPC, S], bf16)  # [k(part), 16 x H]

            # ---- projections ----
            for b in range(BPC):
                for w_sb, o_sb, bias_ap, scale in (
                    (qw_sb, QT, qb_sb, QSCALE),
                    (kw_sb, KT, kb_sb, 1.0),
                ):
                    for cb in range(S // QB):
                        ps = psum_mm.tile([128, QB], f32, tag="mm")
                        for c in range(D // 128):
                            nc.tensor.matmul(
                                ps,
                                lhsT=w_sb[:, c, :],
                                rhs=xts[:, b, c, cb * QB : (cb + 1) * QB],
                                start=(c == 0),
                                stop=(c == D // 128 - 1),
                            )
                        nc.scalar.activation(
                            o_sb[:, b, cb * QB : (cb + 1) * QB],
                            ps,
                            AF.Identity,
                            bias=bias_ap,
                            scale=scale,
                        )
                # V directly in [s, H] layout: lhsT = xT chunk slice
                for t in range(S // 128):
                    psv = psum_tp.tile([128, 128], f32, tag="tp")
                    for c in range(D // 128):
                        nc.tensor.matmul(
                            psv,
                            lhsT=xts[:, b, c, t * 128 : (t + 1) * 128],
                            rhs=vw_sb[:, c, :],
                            start=(c == 0),
                            stop=(c == D // 128 - 1),
                        )
                    nc.vector.tensor_add(
                        Vs[:, b, t * 128 : (t + 1) * 128], psv, vb_sb
                    )

            # ---- attention ----
            for b in range(BPC):
                for qb in range(NQB):
                    dn_ps = psum_acc.tile([128, QB], f32, tag="dn")
                    o_ps = psum_acc.tile([128, QB], f32, tag="o")
                    qs = QT[:, b, qb * QB : (qb + 1) * QB]
                    for kb in range(NKB):
                        sc = psum_mm.tile([128, QB], f32, tag="mm")
                        nc.tensor.matmul(
                            sc,
                            lhsT=KT[:, b, kb * 128 : (kb + 1) * 128],
                            rhs=qs,
                            start=True,
                            stop=True,
                        )
                        E = work.tile([128, QB], bf16, tag="E")
                        nc.scalar.activation(
                            E, sc, AF.Exp, bias=bias_sb[:, b, kb : kb + 1], scale=1.0
                        )
                        nc.tensor.matmul(
                            dn_ps, lhsT=ones, rhs=E,
                            start=(kb == 0), stop=(kb == NKB - 1),
                        )
                        M = mwork.tile([128, QB], bf16, tag="M")
                        nc.sync.dma_start(
                            M,
                            maskT[b, kb * 128 : (kb + 1) * 128,
                                  qb * QB : (qb + 1) * QB],
                        )
                        E2 = work.tile([128, QB], bf16, tag="E2")
                        nc.vector.tensor_mul(E2, E, M)
                        nc.tensor.matmul(
                            o_ps,
                            lhsT=Vs[:, b, kb * 128 : (kb + 1) * 128],
                            rhs=E2,
                            start=(kb == 0),
                            stop=(kb == NKB - 1),
                        )
                    # epilogue: normalize + transpose out to [q, H]
                    dn_sb = ep.tile([128, QB], f32, tag="dn_sb")
                    nc.vector.tensor_copy(dn_sb, dn_ps)
                    oc_sb = ep.tile([128, QB], f32, tag="oc_sb")
                    nc.scalar.copy(oc_sb, o_ps)
                    rT = ep.tile([128, QB // 128], f32, tag="rT")
                    for t in range(QB // 128):
                        dt_ps = psum_tp.tile([128, 128], f32, tag="tp")
                        nc.tensor.transpose(
                            dt_ps, dn_sb[:, t * 128 : (t + 1) * 128], ident
                        )
                        nc.vector.reciprocal(rT[:, t : t + 1], dt_ps[:, 0:1])
                        ot_ps = psum_tp.tile([128, 128], f32, tag="tp")
                        nc.tensor.transpose(
                            ot_ps, oc_sb[:, t * 128 : (t + 1) * 128], ident
                        )
                        of = ep.tile([128, 128], f32, tag="of")
                        nc.scalar.activation(
                            of, ot_ps, AF.Copy, scale=rT[:, t : t + 1]
                        )
                        r0 = qb * QB + t * 128
                        nc.sync.dma_start(out[b, r0 : r0 + 128, :], of)
    nc.compile()
    return nc


def _host_prep(x, attention_mask, qw, qb, kw, kb, vw, vb):
    """Build per-core input maps (list of dicts keyed by dram tensor name)."""
    import jax

    x = np.asarray(x, dtype=np.float32)
    attention_mask = np.asarray(attention_mask)
    qw = np.asarray(qw, dtype=np.float32)
    kw = np.asarray(kw, dtype=np.float32)
    vw = np.asarray(vw, dtype=np.float32)
    qb = np.asarray(qb, dtype=np.float32)
    kb = np.asarray(kb, dtype=np.float32)
    vb = np.asarray(vb, dtype=np.float32)

    # x^T per batch: [B, D, S] bf16
    xT = np.ascontiguousarray(x.transpose(0, 2, 1)).astype(_BF16)

    # dropout keep mask, bit-exact with the reference (fixed key 42), on CPU
    cpu = jax.devices("cpu")[0]
    with jax.default_device(cpu):
        keep = jax.random.bernoulli(
            jax.random.key(42), 1.0 - DROPOUT_P, (B, S, S)
        )
        keep = np.asarray(keep)  # bool [B, q, k]
    keepT = keep.transpose(0, 2, 1)  # [B, k, q]
    scale = _BF16(1.0 / (1.0 - DROPOUT_P))
    maskT = np.where(keepT, scale, _BF16(0.0))  # bf16 [B, k, q]

    # additive attention bias per k: 0 keep, -1e9 masked; layout [B, 128, NKB]
    bias = np.where(attention_mask == 0, np.float32(-1e9), np.float32(0.0))
    bias_r = np.ascontiguousarray(
        bias.reshape(B, NKB, 128).transpose(0, 2, 1)
    ).astype(np.float32)

    qwT = np.ascontiguousarray(qw.T).astype(_BF16)
    kwT = np.ascontiguousarray(kw.T).astype(_BF16)
    vwT = np.ascontiguousarray(vw.T).astype(_BF16)
    qb2 = (qb * QSCALE).reshape(H, 1).astype(np.float32)
    kb2 = kb.reshape(H, 1).astype(np.float32)
    vb2 = vb.reshape(1, H).astype(_BF16)

    in_maps = []
    for c in range(N_CORES):
        lo, hi = c * BPC, (c + 1) * BPC
        in_maps.append(
            dict(
                xT=np.ascontiguousarray(xT[lo:hi]),
                qwT=qwT, kwT=kwT, vwT=vwT,
                qb2=qb2, kb2=kb2, vb2=vb2,
                biask=np.ascontiguousarray(bias_r[lo:hi]),
                maskT=np.ascontiguousarray(maskT[lo:hi]),
            )
        )
    return in_maps


def run(inputs, trace=False, trace_cores=None):
    """Build, run on 8 cores, return (full_output, BassKernelResults)."""
    from concourse.bass_utils import run_bass_kernel_spmd

    in_maps = _host_prep(**inputs)
    nc = _build_bass()
    res = run_bass_kernel_spmd(
        nc,
        in_maps,
        core_ids=list(range(N_CORES)),
        trace=trace,
        trace_cores=trace_cores,
    )
    outs = [r["out"] for r in res.results]
    full = np.concatenate(outs, axis=0).astype(np.float32)
    return full, res


def kernel(**inputs) -> np.ndarray:
    full, _ = run(inputs, trace=False)
    return full


# revision 5
# speedup vs baseline: 1.0388x; 1.0388x over previous
"""Trainium2 Bass kernel for nn_AttentionHead (B=16, S=2048, D=1024, H=128).

Single attention head with key-mask + deterministic dropout (jax key 42).
Data-parallel over batch: 2 batch elements per core on 8 cores.

Per-core layout strategy (all matmuls contract over the partition dim):
  - x is pre-transposed on host to xT [D, S] so QKV projections run directly.
  - Q^T, K^T, V^T produced as [H=128 part, S free] (c-outer accumulation so
    the PE starts as soon as the first xT chunk lands); V^T is transposed
    on the PE to V [k part, H free] for the AV matmul.
  - scores^T [k part, q free] = K_blk @ Q^T  (one 128-contraction matmul).
  - attn mask folds into exp's per-partition bias (-1e9 on masked k).
  - E = exp(scores^T + bias); denominator = ones-matmul over E (PSUM f32).
  - dropout = precomputed {0, 1/(1-p)} bf16 mask, one 2MB DMA per q-block.
  - out^T [H, q] = sum_k V_blk^T @ (E*mask); transpose + per-q 1/denom scale.

qb/kb/vb are zeros per the problem spec (asserted on host) and folded out.
"""

import math

import numpy as np
import ml_dtypes

B, S, D, H = 16, 2048, 1024, 128
N_CORES = 8
BPC = B // N_CORES  # batches per core
QB = 512            # query block (free dim per matmul)
NQB = S // QB
NKB = S // 128      # key blocks of 128
NC = D // 128       # contraction chunks
DROPOUT_P = 0.3
QSCALE = 1.0 / math.sqrt(H)

_BF16 = ml_dtypes.bfloat16


def _build_bass():
    import concourse.mybir as mybir
    import concourse.tile as tile
    from concourse import bacc
    from concourse.masks import make_identity

    f32 = mybir.dt.float32
    bf16 = mybir.dt.bfloat16
    AF = mybir.ActivationFunctionType

    nc = bacc.Bacc(None, target_bir_lowering=False, debug=False)

    xT = nc.dram_tensor("xT", [BPC, D, S], bf16, kind="ExternalInput")
    qwT = nc.dram_tensor("qwT", [D, H], bf16, kind="ExternalInput")
    kwT = nc.dram_tensor("kwT", [D, H], bf16, kind="ExternalInput")
    vwT = nc.dram_tensor("vwT", [D, H], bf16, kind="ExternalInput")
    biask = nc.dram_tensor("biask", [BPC, 128, NKB], f32, kind="ExternalInput")
    maskR = nc.dram_tensor(
        "maskR", [BPC, NQB, 128, NKB, QB], bf16, kind="ExternalInput"
    )
    out = nc.dram_tensor("out", [BPC, S, H], f32, kind="ExternalOutput")

    with tile.TileContext(nc) as tc:
        with (
            tc.tile_pool(name="singles", bufs=1) as singles,
            tc.tile_pool(name="work", bufs=4) as work,
            tc.tile_pool(name="mwork", bufs=2) as mwork,
            tc.tile_pool(name="ep", bufs=2) as ep,
            tc.tile_pool(name="psum_mm", bufs=2, space="PSUM") as psum_mm,
            tc.tile_pool(name="psum_acc", bufs=4, space="PSUM") as psum_acc,
            tc.tile_pool(name="psum_tp", bufs=2, space="PSUM") as psum_tp,
        ):
            # ---- constants / weights ----
            ident = singles.tile([128, 128], f32)
            make_identity(nc, ident)
            identb = singles.tile([128, 128], bf16)
            make_identity(nc, identb)
            ones = singles.tile([128, 128], bf16)
            nc.vector.memset(ones, 1.0)

            qw_sb = singles.tile([128, NC, H], bf16)
            kw_sb = singles.tile([128, NC, H], bf16)
            vw_sb = singles.tile([128, NC, H], bf16)
            nc.sync.dma_start(qw_sb, qwT.rearrange("(c p) h -> p c h", p=128))
            nc.sync.dma_start(kw_sb, kwT.rearrange("(c p) h -> p c h", p=128))
            nc.sync.dma_start(vw_sb, vwT.rearrange("(c p) h -> p c h", p=128))

            bias_sb = singles.tile([128, BPC, NKB], f32)
            nc.sync.dma_start(bias_sb, biask.rearrange("b p t -> p b t"))

            # ---- x^T resident in SBUF: [128, BPC, NC, S] bf16 ----
            xts = singles.tile([128, BPC, NC, S], bf16)
            for b in range(BPC):
                for c in range(NC):
                    nc.sync.dma_start(
                        xts[:, b, c, :], xT[b, c * 128 : (c + 1) * 128, :]
                    )

            QT = singles.tile([128, BPC, S], bf16)  # [H, q]
            KT = singles.tile([128, BPC, S], bf16)  # [H, k]
            Vs = singles.tile([128, BPC, S], bf16)  # [k(part), 16 x H]

            # ---- projections (c-outer: start matmuls on first x chunk) ----
            for b in range(BPC):
                for w_sb, o_sb, scale in (
                    (qw_sb, QT, QSCALE),
                    (kw_sb, KT, 1.0),
                ):
                    pss = [
                        psum_acc.tile([128, QB], f32, tag="acc", name=f"acc{i}")
                        for i in range(NQB)
                    ]
                    for c in range(NC):
                        for cb in range(NQB):
                            nc.tensor.matmul(
                                pss[cb],
                                lhsT=w_sb[:, c, :],
                                rhs=xts[:, b, c, cb * QB : (cb + 1) * QB],
                                start=(c == 0),
                                stop=(c == NC - 1),
                            )
                    for cb in range(NQB):
                        nc.scalar.activation(
                            o_sb[:, b, cb * QB : (cb + 1) * QB],
                            pss[cb],
                            AF.Copy,
                            scale=scale,
                        )
                # V^T then PE-transpose into [k, H] layout
                vts = work.tile([128, S], bf16, tag="VT")
                pss = [
                    psum_acc.tile([128, QB], f32, tag="acc", name=f"acc{i}")
                    for i in range(NQB)
                ]
                for c in range(NC):
                    for cb in range(NQB):
                        nc.tensor.matmul(
                            pss[cb],
                            lhsT=vw_sb[:, c, :],
                            rhs=xts[:, b, c, cb * QB : (cb + 1) * QB],
                            start=(c == 0),
                            stop=(c == NC - 1),
                        )
                for cb in range(NQB):
                    nc.scalar.activation(
                        vts[:, cb * QB : (cb + 1) * QB], pss[cb], AF.Copy
                    )
                for t in range(S // 128):
                    ptv = psum_tp.tile([128, 128], bf16, tag="tp")
                    nc.tensor.transpose(
                        ptv, vts[:, t * 128 : (t + 1) * 128], identb
                    )
                    nc.vector.tensor_copy(Vs[:, b, t * 128 : (t + 1) * 128], ptv)

            # ---- attention ----
            for b in range(BPC):
                for qb in range(NQB):
                    Mq = mwork.tile([128, NKB, QB], bf16, tag="M")
                    nc.sync.dma_start(Mq, maskR[b, qb])
                    dn_ps = psum_acc.tile([128, QB], f32, tag="acc")
                    o_ps = psum_acc.tile([128, QB], f32, tag="acc")
                    qs = QT[:, b, qb * QB : (qb + 1) * QB]
                    for kb in range(NKB):
                        sc = psum_mm.tile([128, QB], f32, tag="mm")
                        nc.tensor.matmul(
                            sc,
                            lhsT=KT[:, b, kb * 128 : (kb + 1) * 128],
                            rhs=qs,
                            start=True,
                            stop=True,
                        )
                        E = work.tile([128, QB], bf16, tag="E")
                        nc.scalar.activation(
                            E, sc, AF.Exp, bias=bias_sb[:, b, kb : kb + 1], scale=1.0
                        )
                        nc.tensor.matmul(
                            dn_ps, lhsT=ones, rhs=E,
                            start=(kb == 0), stop=(kb == NKB - 1),
                        )
                        E2 = work.tile([128, QB], bf16, tag="E2")
                        nc.vector.tensor_mul(E2, E, Mq[:, kb, :])
                        nc.tensor.matmul(
                            o_ps,
                            lhsT=Vs[:, b, kb * 128 : (kb + 1) * 128],
                            rhs=E2,
                            start=(kb == 0),
                            stop=(kb == NKB - 1),
                        )
                    # epilogue: normalize + transpose out to [q, H]
                    dn_sb = ep.tile([128, QB], f32, tag="dn_sb")
                    nc.vector.tensor_copy(dn_sb, dn_ps)
                    oc_sb = ep.tile([128, QB], f32, tag="oc_sb")
                    nc.scalar.copy(oc_sb, o_ps)
                    rT = ep.tile([128, QB // 128], f32, tag="rT")
                    for t in range(QB // 128):
                        dt_ps = psum_tp.tile([128, 128], f32, tag="tp")
                        nc.tensor.transpose(
                            dt_ps, dn_sb[:, t * 128 : (t + 1) * 128], ident
                        )
                        nc.vector.reciprocal(rT[:, t : t + 1], dt_ps[:, 0:1])
                        ot_ps = psum_tp.tile([128, 128], f32, tag="tp")
                        nc.tensor.transpose(
                            ot_ps, oc_sb[:, t * 128 : (t + 1) * 128], ident
                        )
                        of = ep.tile([128, 128], f32, tag="of")
                        nc.vector.tensor_scalar_mul(of, ot_ps, rT[:, t : t + 1])
                        r0 = qb * QB + t * 128
                        nc.sync.dma_start(out[b, r0 : r0 + 128, :], of)
    nc.compile()
    return nc


def _host_prep(x, attention_mask, qw, qb, kw, kb, vw, vb):
    """Build per-core input maps (list of dicts keyed by dram tensor name)."""
    import jax

    x = np.asarray(x, dtype=np.float32)
    attention_mask = np.asarray(attention_mask)
    qw = np.asarray(qw, dtype=np.float32)
    kw = np.asarray(kw, dtype=np.float32)
    vw = np.asarray(vw, dtype=np.float32)
    for name, bias in (("qb", qb), ("kb", kb), ("vb", vb)):
        assert not np.any(np.asarray(bias)), f"{name} expected to be zero"

    # x^T per batch: [B, D, S] bf16
    xT = np.ascontiguousarray(x.transpose(0, 2, 1)).astype(_BF16)

    # dropout keep mask, bit-exact with the reference (fixed key 42), on CPU
    cpu = jax.devices("cpu")[0]
    with jax.default_device(cpu):
        keep = jax.random.bernoulli(
            jax.random.key(42), 1.0 - DROPOUT_P, (B, S, S)
        )
        keep = np.asarray(keep)  # bool [B, q, k]
    keepT = keep.transpose(0, 2, 1)  # [B, k, q]
    scale = _BF16(1.0 / (1.0 - DROPOUT_P))
    maskT = np.where(keepT, scale, _BF16(0.0))  # bf16 [B, k, q]
    # regroup per (qblock): [B, NQB, 128(k mod), NKB, QB] with contiguous
    # 16KB-per-partition runs for single-DMA loading
    maskR = np.ascontiguousarray(
        maskT.reshape(B, NKB, 128, NQB, QB).transpose(0, 3, 2, 1, 4)
    )

    # additive attention bias per k: 0 keep, -1e9 masked; layout [B, 128, NKB]
    bias = np.where(attention_mask == 0, np.float32(-1e9), np.float32(0.0))
    bias_r = np.ascontiguousarray(
        bias.reshape(B, NKB, 128).transpose(0, 2, 1)
    ).astype(np.float32)

    qwT = np.ascontiguousarray(qw.T).astype(_BF16)
    kwT = np.ascontiguousarray(kw.T).astype(_BF16)
    vwT = np.ascontiguousarray(vw.T).astype(_BF16)

    in_maps = []
    for c in range(N_CORES):
        lo, hi = c * BPC, (c + 1) * BPC
        in_maps.append(
            dict(
                xT=np.ascontiguousarray(xT[lo:hi]),
                qwT=qwT, kwT=kwT, vwT=vwT,
                biask=np.ascontiguousarray(bias_r[lo:hi]),
                maskR=np.ascontiguousarray(maskR[lo:hi]),
            )
        )
    return in_maps


def run(inputs, trace=False, trace_cores=None):
    """Build, run on 8 cores, return (full_output, BassKernelResults)."""
    from concourse.bass_utils import run_bass_kernel_spmd

    in_maps = _host_prep(**inputs)
    nc = _build_bass()
    res = run_bass_kernel_spmd(
        nc,
        in_maps,
        core_ids=list(range(N_CORES)),
        trace=trace,
        trace_cores=trace_cores,
    )
    outs = [r["out"] for r in res.results]
    full = np.concatenate(outs, axis=0).astype(np.float32)
    return full, res


def kernel(**inputs) -> np.ndarray:
    full, _ = run(inputs, trace=False)
    return full
